# revision 1
# baseline (speedup 1.0000x reference)
"""GCEncoder (RGCN basis-decomposition conv + mean aggregation + Dense/BN/ReLU)
as a Bass/Tile kernel on 8 Trainium2 NeuronCores.

Math (reference):
  W[r]  = sum_b comp[r,b] * basis[b]                    [R, N, H0]
  h[r]  = x @ W[r]                                      [R, N, H0]
  agg[d] = sum_r (1/cnt[d,r]) * sum_{e: dst=d, type=r} h[r, src_e]
  feats = agg + x @ root + bias
  z     = feats @ fc_w.T ; per-row batchnorm over H1 + gamma/beta + relu
  out   = (z[:U], z[U:]) stacked -> [2, U, H1]

Device strategy (per core c of 8, 512 node-rows each):
  Phase A: h rows for this core's 512 src rows: h_c = x[rows] @ Wall where
           Wall = [W[0] | ... | W[4] | root]  (4096 x 3000).  The root block
           result stays local in fp32 (these rows are exactly this core's dst
           rows); each relation block r is AllGathered as soon as it is done
           (5 chunked collectives overlap with the remaining compute).
  Phase B: agg rows via dense normalized-adjacency matmul: contraction over
           the 20480 (r,src) axis with host-built AT[(r,src), dst_local],
           PSUM-accumulated across 160 k-tiles into 4 persistent banks.
  Phase C: feats = agg + root_part + bias; PE-transpose; z = feats @ fc_w.T;
           per-row BN (bn_stats/bn_aggr) + gamma/beta + ReLU.

Matmul operands are bf16 (fp32 PSUM accumulation); set USE_FP32R=True for
E8M11 fp32r operands instead (2x slower matmul stream + 2x DMA, ~15x lower
error).  All heavy inputs are host-pre-swizzled so each DMA lands >=4KB
contiguous per SBUF partition.
"""
import numpy as np
import ml_dtypes

import concourse.bacc as bacc
import concourse.mybir as mybir
import concourse.tile as tile
from concourse.bass_utils import run_bass_kernel_spmd
from concourse.masks import make_identity

P = 128
NCORES = 8
N = 4096          # nodes
U = 2048          # users
R = 5             # relations
H0 = 500
H1 = 75
EPS = 1e-5

NL = N // NCORES              # 512 node rows per core
KB_A = N // P                 # 32 contraction tiles, phase A
WCOL = R * H0 + H0            # 3000 Wall columns
NBLK = WCOL // H0             # 6 column blocks of 500
MB = NL // P                  # 4 M-tiles per core
QB = 4                        # H0 chunks for transpose/fc
QS = H0 // QB                 # 125

F32 = mybir.dt.float32

USE_FP32R = False
if USE_FP32R:
    DT_MM = mybir.dt.float32r
else:
    DT_MM = mybir.dt.bfloat16

# test hooks
TRACE = False
LAST_RESULTS = None
_NC_CACHE = None


def round_fp32r(a: np.ndarray) -> np.ndarray:
    """Round fp32 to fp32r (E8M11): RNE at mantissa bit 12, low 12 bits zero."""
    b = np.ascontiguousarray(a, dtype=np.float32).view(np.uint32).astype(np.uint64)
    b = b + 0x7FF + ((b >> 12) & 1)
    return (b & 0xFFFFF000).astype(np.uint32).view(np.float32)


def _prep_mm(a: np.ndarray) -> np.ndarray:
    """Convert host fp32 data to the matmul operand dtype."""
    if USE_FP32R:
        return round_fp32r(a)
    return np.ascontiguousarray(a).astype(ml_dtypes.bfloat16)


def _build():
    nc = bacc.Bacc("TRN2", target_bir_lowering=False, debug=False,
                   num_devices=NCORES)

    # host-swizzled inputs; layouts noted as [partition, free...]
    # x4[p, kb*NL + m] = x[coreRows m][i = kb*128+p]
    x4_d = nc.dram_tensor("x4", [P, KB_A * NL], DT_MM, kind="ExternalInput")
    # w4[p, ((n*32+kb) * H0) + j] = Wall[kb*128+p, n*500+j]
    w4_d = nc.dram_tensor("w4", [P, NBLK * KB_A * H0], DT_MM,
                          kind="ExternalInput")
    # a4[p, kb*NL + d] = AT[kb*128+p, d]   (kb = r*32 + cb*4 + mk)
    a4_d = nc.dram_tensor("a4", [P, R * KB_A * NL], DT_MM,
                          kind="ExternalInput")
    fcwt_d = nc.dram_tensor("fcwt", [H0, H1], F32, kind="ExternalInput")
    biasb_d = nc.dram_tensor("biasb", [P, H0], F32, kind="ExternalInput")
    gamma_d = nc.dram_tensor("gamma", [P, MB], F32, kind="ExternalInput")
    beta_d = nc.dram_tensor("beta", [P, MB], F32, kind="ExternalInput")
    out_d = nc.dram_tensor("out", [NL, H1], F32, kind="ExternalOutput")

    with tile.TileContext(nc) as tc:
        with (
            tc.tile_pool(name="big", bufs=1) as big,
            tc.tile_pool(name="slab", bufs=3) as slabp,
            tc.tile_pool(name="io", bufs=4) as iop,
            tc.tile_pool(name="bstream", bufs=4) as bsp,
            tc.tile_pool(name="persist", bufs=4) as pp,
            tc.tile_pool(name="bn", bufs=4) as bnp,
            tc.tile_pool(name="ps", bufs=4, space="PSUM") as psp,
            tc.tile_pool(name="dram", bufs=1, space="DRAM") as dramp,
        ):
            # ---------------- Phase A: h_c = x_rows @ Wall ----------------
            pre_slab = slabp.tile([P, KB_A // 2, H0], DT_MM, tag="slab",
                                  name="slab00")
            nc.scalar.dma_start(out=pre_slab, in_=w4_d[:, :16 * H0])
            xt_sb = big.tile([P, KB_A, NL], DT_MM, tag="xt")
            for ch in range(4):
                eng = nc.sync if ch < 2 else nc.scalar
                eng.dma_start(
                    out=xt_sb[:, ch * 8:(ch + 1) * 8, :],
                    in_=x4_d[:, ch * 8 * NL:(ch + 1) * 8 * NL],
                )

            # per-relation h buffers: h_cr[p, m*500+j]; gathered to
            # h_ar[128*rank + p, m*500+j]
            h_cr = [dramp.tile([P, MB * H0], DT_MM, tag="h_c", name=f"h_c{r}")
                    for r in range(R)]
            h_ar = [dramp.tile([NCORES * P, MB * H0], DT_MM, tag="h_a",
                               addr_space="Shared", name=f"h_a{r}")
                    for r in range(R)]

            rootf = []
            for n in range(NBLK):
                ps_n = [psp.tile([P, H0], F32, tag="psA",
                                 name=f"psA_{n}_{m}") for m in range(MB)]
                for kh in range(2):
                    if n == 0 and kh == 0:
                        slab = pre_slab
                    else:
                        slab = slabp.tile([P, KB_A // 2, H0], DT_MM,
                                          tag="slab")
                        base = (n * KB_A + kh * 16) * H0
                        nc.sync.dma_start(
                            out=slab,
                            in_=w4_d[:, base:base + 16 * H0],
                        )
                    for k in range(KB_A // 2):
                        kb = kh * 16 + k
                        for m in range(MB):
                            nc.tensor.matmul(
                                ps_n[m],
                                xt_sb[:, kb, m * P:(m + 1) * P],
                                slab[:, k, :],
                                start=(kb == 0),
                                stop=(kb == KB_A - 1),
                            )
                for m in range(MB):
                    if n == NBLK - 1:
                        rf = pp.tile([P, H0], F32, tag="rootf",
                                     name=f"rootf_{m}")
                        nc.vector.tensor_copy(out=rf, in_=ps_n[m])
                        rootf.append(rf)
                    else:
                        hsb = iop.tile([P, H0], DT_MM, tag="hout")
                        nc.vector.tensor_copy(out=hsb, in_=ps_n[m])
                        nc.scalar.dma_start(
                            out=h_cr[n][:, m * H0:(m + 1) * H0],
                            in_=hsb,
                        )
                if n < R:
                    nc.gpsimd.collective_compute(
                        "AllGather",
                        mybir.AluOpType.bypass,
                        replica_groups=[list(range(NCORES))],
                        ins=[h_cr[n][:, :]],
                        outs=[h_ar[n][:, :]],
                    )

            # ---------------- Phase B: agg = AT.T-contract @ h ------------
            psB = [psp.tile([P, H0], F32, tag="psB", name=f"psB_{m}")
                   for m in range(MB)]
            for r in range(R):
                for cb in range(NCORES):
                    # share the slab pool's slots: the WAR on slot reuse
                    # keeps this AG-dependent load from being hoisted into
                    # phase A's queue (head-of-line / clock entanglement)
                    hh = slabp.tile([P, MB * H0], DT_MM, tag="slab",
                                    name=f"hh_{r}_{cb}")
                    nc.gpsimd.dma_start(
                        out=hh, in_=h_ar[r][cb * P:(cb + 1) * P, :]
                    )
                    aa = bsp.tile([P, MB, NL], DT_MM, tag="aa")
                    base = (r * KB_A + cb * MB) * NL
                    nc.sync.dma_start(
                        out=aa, in_=a4_d[:, base:base + MB * NL]
                    )
                    first = (r == 0 and cb == 0)
                    last = (r == R - 1 and cb == NCORES - 1)
                    for mk in range(MB):
                        for m in range(MB):
                            nc.tensor.matmul(
                                psB[m],
                                aa[:, mk, m * P:(m + 1) * P],
                                hh[:, mk * H0:(mk + 1) * H0],
                                start=(first and mk == 0),
                                stop=(last and mk == MB - 1),
                            )

            # ---------------- Phase C: feats -> fc -> BN -> ReLU ----------
            fcw_sb = big.tile([QS, QB, H1], F32, tag="fcw")
            nc.scalar.dma_start(
                out=fcw_sb,
                in_=fcwt_d[:, :].rearrange("(q p) j -> p q j", p=QS),
            )
            ident = big.tile([P, P], F32, tag="ident")
            make_identity(nc, ident)
            biasb = big.tile([P, H0], F32, tag="bias")
            nc.scalar.dma_start(out=biasb, in_=biasb_d[:, :])
            gam = big.tile([P, MB], F32, tag="gam")
            nc.scalar.dma_start(out=gam, in_=gamma_d[:, :])
            bet = big.tile([P, MB], F32, tag="bet")
            nc.scalar.dma_start(out=bet, in_=beta_d[:, :])
            eps_t = big.tile([P, 1], F32, tag="eps")
            nc.vector.memset(eps_t, EPS)

            feats = []
            for m in range(MB):
                f = pp.tile([P, H0], F32, tag="feats", name=f"feats_{m}")
                nc.vector.tensor_add(out=f, in0=psB[m], in1=rootf[m])
                nc.vector.tensor_add(out=f, in0=f, in1=biasb)
                feats.append(f)

            fT = [pp.tile([P, NL], F32, tag="fT", name=f"fT_{q}")
                  for q in range(QB)]
            for m in range(MB):
                for q in range(QB):
                    pt = psp.tile([P, P], F32, tag="psA", name=f"pt_{m}_{q}")
                    nc.tensor.transpose(
                        pt[:QS, :], feats[m][:, q * QS:(q + 1) * QS], ident
                    )
                    nc.vector.tensor_copy(
                        out=fT[q][:QS, m * P:(m + 1) * P], in_=pt[:QS, :]
                    )

            for m in range(MB):
                pz = psp.tile([P, H1], F32, tag="psA", name=f"pz_{m}")
                for q in range(QB):
                    nc.tensor.matmul(
                        pz,
                        fT[q][:QS, m * P:(m + 1) * P],
                        fcw_sb[:, q, :],
                        start=(q == 0),
                        stop=(q == QB - 1),
                    )
                stats = bnp.tile([P, 6], F32, tag="stats")
                nc.vector.bn_stats(out=stats, in_=pz)
                mv = bnp.tile([P, 2], F32, tag="mv")
                nc.vector.bn_aggr(out=mv, in_=stats)
                rstd = bnp.tile([P, 1], F32, tag="rstd")
                nc.scalar.activation(
                    out=rstd, in_=mv[:, 1:2],
                    func=mybir.ActivationFunctionType.Sqrt,
                    bias=eps_t, scale=1.0,
                )
                nc.vector.reciprocal(out=rstd, in_=rstd)
                g2 = bnp.tile([P, 1], F32, tag="g2")
                nc.vector.tensor_mul(out=g2, in0=rstd, in1=gam[:, m:m + 1])
                zt = bnp.tile([P, H1], F32, tag="zt")
                nc.vector.tensor_scalar(
                    out=zt, in0=pz,
                    scalar1=mv[:, 0:1], scalar2=g2,
                    op0=mybir.AluOpType.subtract, op1=mybir.AluOpType.mult,
                )
                nc.scalar.activation(
                    out=zt, in_=zt,
                    func=mybir.ActivationFunctionType.Relu,
                    bias=bet[:, m:m + 1], scale=1.0,
                )
                nc.scalar.dma_start(out=out_d[m * P:(m + 1) * P, :], in_=zt)

    nc.finalize()
    return nc


def _get_nc():
    global _NC_CACHE
    if _NC_CACHE is None:
        _NC_CACHE = _build()
    return _NC_CACHE


def kernel(**inputs) -> np.ndarray:
    global LAST_RESULTS
    x = np.asarray(inputs["x"], dtype=np.float32)
    basis = np.asarray(inputs["basis"], dtype=np.float32)
    comp = np.asarray(inputs["comp"], dtype=np.float32)
    root = np.asarray(inputs["root"], dtype=np.float32)
    bias_rgcn = np.asarray(inputs["bias_rgcn"], dtype=np.float32)
    fc_w = np.asarray(inputs["fc_w"], dtype=np.float32)
    bn_gamma_u = np.asarray(inputs["bn_gamma_u"], dtype=np.float32)
    bn_beta_u = np.asarray(inputs["bn_beta_u"], dtype=np.float32)
    bn_gamma_i = np.asarray(inputs["bn_gamma_i"], dtype=np.float32)
    bn_beta_i = np.asarray(inputs["bn_beta_i"], dtype=np.float32)
    edge_index = np.asarray(inputs["edge_index"]).astype(np.int64)
    edge_type = np.asarray(inputs["edge_type"]).astype(np.int64)

    src, dst = edge_index[0], edge_index[1]
    et = edge_type

    # W[r] = sum_b comp[r,b] basis[b]; Wall = [W | root]
    W = np.tensordot(comp, basis, axes=([1], [0]))          # [R, N, H0]
    wall = np.empty((N, WCOL), dtype=np.float32)
    wall[:, :R * H0] = W.transpose(1, 0, 2).reshape(N, R * H0)
    wall[:, R * H0:] = root
    wall16 = _prep_mm(wall)
    # w4[p, (n*32+kb)*H0 + j] = wall[kb*128+p, n*500+j]
    w4 = np.ascontiguousarray(
        wall16.reshape(KB_A, P, NBLK, H0)       # [kb, p, n, j]
        .transpose(1, 2, 0, 3)                  # [p, n, kb, j]
        .reshape(P, NBLK * KB_A * H0))

    xT16 = _prep_mm(x.T)                                    # [i, s]
    # x4[p, kb*NL + m] = x.T[kb*128+p, m@core]  (per-core slice below)
    x4_full = (xT16.reshape(KB_A, P, N)         # [kb, p, s]
               .transpose(1, 0, 2))             # [p, kb, s]

    # normalized adjacency transposed: AT[(r*N+src), dst] = count/cnt[dst,r]
    cnt = np.bincount(dst * R + et, minlength=N * R).astype(np.float64)
    w_e = 1.0 / np.maximum(cnt[dst * R + et], 1.0)
    lin = (et * N + src) * np.int64(N) + dst
    at_full = np.bincount(lin, weights=w_e, minlength=R * N * N)
    at_full = _prep_mm(at_full.astype(np.float32).reshape(R * N, N))
    # a4[p, kb*NL + d] = AT[kb*128+p, d]
    a4_full = (at_full.reshape(R * KB_A, P, N)  # [kb, p, d]
               .transpose(1, 0, 2))             # [p, kb, d]

    fcwt = np.ascontiguousarray(fc_w.T)
    biasb = np.ascontiguousarray(
        np.broadcast_to(bias_rgcn, (P, H0)), dtype=np.float32)
    gamma_all = np.concatenate([bn_gamma_u, bn_gamma_i])
    beta_all = np.concatenate([bn_beta_u, bn_beta_i])

    in_maps = []
    for c in range(NCORES):
        sl = slice(c * NL, (c + 1) * NL)
        in_maps.append({
            "x4": np.ascontiguousarray(
                x4_full[:, :, sl]).reshape(P, KB_A * NL),
            "w4": w4,
            "a4": np.ascontiguousarray(
                a4_full[:, :, sl]).reshape(P, R * KB_A * NL),
            "fcwt": fcwt,
            "biasb": biasb,
            "gamma": np.ascontiguousarray(gamma_all[sl].reshape(MB, P).T),
            "beta": np.ascontiguousarray(beta_all[sl].reshape(MB, P).T),
        })

    nc = _get_nc()
    res = run_bass_kernel_spmd(
        nc, in_maps, core_ids=list(range(NCORES)), trace=TRACE,
    )
    LAST_RESULTS = res

    z = np.concatenate([res.results[c]["out"] for c in range(NCORES)], axis=0)
    return np.stack([z[:U], z[U:]], axis=0)



# revision 2
# speedup vs baseline: 1.3606x; 1.3606x over previous
"""GCEncoder (RGCN basis-decomposition conv + mean aggregation + Dense/BN/ReLU)
as a Bass/Tile kernel on 8 Trainium2 NeuronCores.

Math (reference):
  W[r]  = sum_b comp[r,b] * basis[b]                    [R, N, H0]
  h[r]  = x @ W[r]                                      [R, N, H0]
  agg[d] = sum_r (1/cnt[d,r]) * sum_{e: dst=d, type=r} h[r, src_e]
  feats = agg + x @ root + bias
  z     = feats @ fc_w.T ; per-row batchnorm over H1 + gamma/beta + relu
  out   = (z[:U], z[U:]) stacked -> [2, U, H1]

Device strategy (per core c of 8, 512 node-rows each), fp8-DoubleRow:
  Phase A: h_c = x[rows] @ W[r] for r=0..4 as fp8e4 DoubleRow matmuls
           (256-deep contraction per instruction, 2x bf16 rate); the
           root block x[rows] @ root stays bf16 for accuracy.  Each
           relation's h block is scaled to fp8 range, drained to fp8
           and AllGathered as soon as it is done.
  Phase B: agg via dense 0/1-multiplicity adjacency matmul in fp8
           DoubleRow (adjacency counts are exact in fp8).  Per-relation
           PSUM accumulation; the 1/cnt mean normalization is applied
           in exact fp32 on the vector engine when each relation's
           accumulation group completes.
  Phase C: feats = agg + root_part + bias; PE-transpose; z = feats @
           fc_w.T; per-row BN (bn_stats/bn_aggr) + gamma/beta + ReLU.

All fp8 scales are powers of two so scaling/descale is exact.  H0=500
is padded to 512 everywhere (zero padding) so DoubleRow pair-steps are
16B-aligned and PSUM tiles are exactly one bank.
"""
import numpy as np
import ml_dtypes

import concourse.bacc as bacc
import concourse.mybir as mybir
import concourse.tile as tile
from concourse.bass_utils import run_bass_kernel_spmd
from concourse.masks import make_identity

P = 128
NCORES = 8
N = 4096          # nodes
U = 2048          # users
R = 5             # relations
H0 = 500
H0P = 512         # padded H0
H1 = 75
EPS = 1e-5

NL = N // NCORES              # 512 node rows per core
KB2 = N // 256                # 16 DoubleRow contraction blocks (phase A)
KBR = N // P                  # 32 bf16 contraction blocks (root)
MB = NL // P                  # 4 M-tiles per core
QB = 4                        # H0 chunks for transpose/fc
QS = H0 // QB                 # 125

F32 = mybir.dt.float32
BF16 = mybir.dt.bfloat16
FP8 = mybir.dt.float8e4
DR = mybir.MatmulPerfMode.DoubleRow
NP8 = ml_dtypes.float8_e4m3   # TRN fp8_e4m3 (bias 7, max 240)

# test hooks
TRACE = False
LAST_RESULTS = None
_NC_CACHE = None


def _pow2_scale(absmax: float, cap: float) -> float:
    return float(2.0 ** np.floor(np.log2(cap / max(absmax, 1e-30))))


def _q8(a: np.ndarray, scale: float) -> np.ndarray:
    return np.clip(a * scale, -240.0, 240.0).astype(NP8)


def _build(ds_const: float):
    nc = bacc.Bacc("TRN2", target_bir_lowering=False, debug=False,
                   num_devices=NCORES)

    # host-swizzled inputs; layouts noted as [partition, free...]
    # x8[p, kb2*1024 + i*512 + s] = q8(x[rows s][k=kb2*256+i*128+p] * SX)
    x8_d = nc.dram_tensor("x8", [P, KB2 * 2 * NL], FP8, kind="ExternalInput")
    # w8[p, ((n*16+kb2)*2+i)*512 + j] = q8(W[n][kb2*256+i*128+p, j] * SW)
    w8_d = nc.dram_tensor("w8", [P, R * KB2 * 2 * H0P], FP8,
                          kind="ExternalInput")
    # xb[p, kb*512 + s] = bf16(x[rows s][kb*128+p])
    xb_d = nc.dram_tensor("xb", [P, KBR * NL], BF16, kind="ExternalInput")
    # rw[p, kb*512 + j] = bf16(root[kb*128+p, j])  (j >= 500 zero)
    rw_d = nc.dram_tensor("rw", [P, KBR * H0P], BF16, kind="ExternalInput")
    # a8[p, (((r*8+cb)*2+mk)*2+i)*512 + d] =
    #   edgecount[r, src=cb*512+(2mk+i)*128+p, dst=core*512+d]
    a8_d = nc.dram_tensor("a8", [P, R * NCORES * 2 * 2 * NL], FP8,
                          kind="ExternalInput")
    # cinv[p, r*4+m] = 1/(max(cnt[core*512+m*128+p, r],1) * SH)
    cinv_d = nc.dram_tensor("cinv", [P, R * MB], F32, kind="ExternalInput")
    fcwt_d = nc.dram_tensor("fcwt", [H0, H1], F32, kind="ExternalInput")
    biasb_d = nc.dram_tensor("biasb", [P, H0], F32, kind="ExternalInput")
    gamma_d = nc.dram_tensor("gamma", [P, MB], F32, kind="ExternalInput")
    beta_d = nc.dram_tensor("beta", [P, MB], F32, kind="ExternalInput")
    out_d = nc.dram_tensor("out", [NL, H1], F32, kind="ExternalOutput")

    with tile.TileContext(nc) as tc:
        with (
            tc.tile_pool(name="big", bufs=1) as big,
            tc.tile_pool(name="slab", bufs=3) as slabp,
            tc.tile_pool(name="io", bufs=2) as iop,
            tc.tile_pool(name="bstream", bufs=4) as bsp,
            tc.tile_pool(name="persist", bufs=4) as pp,
            tc.tile_pool(name="bn", bufs=4) as bnp,
            tc.tile_pool(name="ps", bufs=8, space="PSUM") as psp,
            tc.tile_pool(name="dram", bufs=1, space="DRAM") as dramp,
        ):
            # ---------------- Phase A: h[r] = x_rows @ W[r] (fp8 DR) ------
            pre_slab = slabp.tile([P, KB2, 2, H0P], FP8, tag="slab",
                                  name="slab00")
            nc.scalar.dma_start(out=pre_slab, in_=w8_d[:, :KB2 * 2 * H0P])
            x8_sb = big.tile([P, KB2, 2, NL], FP8, tag="x8")
            for ch in range(4):
                eng = nc.sync if ch < 2 else nc.scalar
                eng.dma_start(
                    out=x8_sb[:, ch * 4:(ch + 1) * 4, :, :],
                    in_=x8_d[:, ch * 4 * 2 * NL:(ch + 1) * 4 * 2 * NL],
                )
            xb_sb = big.tile([P, KBR, NL], BF16, tag="xb")
            for ch in range(4):
                nc.scalar.dma_start(
                    out=xb_sb[:, ch * 8:(ch + 1) * 8, :],
                    in_=xb_d[:, ch * 8 * NL:(ch + 1) * 8 * NL],
                )

            h_cr = [dramp.tile([P, MB * H0P], FP8, tag="h_c", name=f"h_c{r}")
                    for r in range(R)]
            h_ar = [dramp.tile([NCORES * P, MB * H0P], FP8, tag="h_a",
                               addr_space="Shared", name=f"h_a{r}")
                    for r in range(R)]

            for n in range(R):
                if n == 0:
                    slab = pre_slab
                else:
                    slab = slabp.tile([P, KB2, 2, H0P], FP8, tag="slab")
                    base = n * KB2 * 2 * H0P
                    nc.sync.dma_start(
                        out=slab, in_=w8_d[:, base:base + KB2 * 2 * H0P]
                    )
                ps_n = [psp.tile([P, H0P], F32, tag="ps",
                                 name=f"psA_{n}_{m}") for m in range(MB)]
                for kb2 in range(KB2):
                    for m in range(MB):
                        nc.tensor.matmul(
                            ps_n[m],
                            x8_sb[:, kb2, :, m * P:(m + 1) * P],
                            slab[:, kb2, :, :],
                            start=(kb2 == 0),
                            stop=(kb2 == KB2 - 1),
                            perf_mode=DR,
                        )
                h8blk = iop.tile([P, MB, H0P], FP8, tag="hout")
                for m in range(MB):
                    # h8 = psum * SH/(SX*SW), cast to fp8
                    nc.vector.tensor_scalar(
                        out=h8blk[:, m, :], in0=ps_n[m],
                        scalar1=ds_const, scalar2=None,
                        op0=mybir.AluOpType.mult,
                    )
                nc.scalar.dma_start(out=h_cr[n][:, :], in_=h8blk)
                nc.gpsimd.collective_compute(
                    "AllGather",
                    mybir.AluOpType.bypass,
                    replica_groups=[list(range(NCORES))],
                    ins=[h_cr[n][:, :]],
                    outs=[h_ar[n][:, :]],
                )

            # ---------------- Phase A2: root block (bf16) -----------------
            rootf = []
            ps_rt = [psp.tile([P, H0P], F32, tag="ps", name=f"psR_{m}")
                     for m in range(MB)]
            for kh in range(2):
                rslab = slabp.tile([P, KBR // 2, H0P], BF16, tag="slab",
                                   name=f"rslab{kh}")
                base = kh * (KBR // 2) * H0P
                nc.sync.dma_start(
                    out=rslab, in_=rw_d[:, base:base + (KBR // 2) * H0P]
                )
                for k in range(KBR // 2):
                    kb = kh * (KBR // 2) + k
                    for m in range(MB):
                        nc.tensor.matmul(
                            ps_rt[m],
                            xb_sb[:, kb, m * P:(m + 1) * P],
                            rslab[:, k, :],
                            start=(kb == 0),
                            stop=(kb == KBR - 1),
                        )
            for m in range(MB):
                rf = pp.tile([P, H0P], F32, tag="rootf", name=f"rootf_{m}")
                nc.vector.tensor_copy(out=rf, in_=ps_rt[m])
                rootf.append(rf)

            # ---------------- Phase B: agg (fp8 DR, per-relation) ---------
            cinv = big.tile([P, R * MB], F32, tag="cinv")
            nc.scalar.dma_start(out=cinv, in_=cinv_d[:, :])
            acc = [pp.tile([P, H0P], F32, tag="acc", name=f"acc_{m}")
                   for m in range(MB)]
            for r in range(R):
                ps_r = [psp.tile([P, H0P], F32, tag="ps",
                                 name=f"psB_{r}_{m}") for m in range(MB)]
                for cb in range(NCORES):
                    # share the slab pool's slots: the WAR on slot reuse
                    # keeps this AG-dependent load from being hoisted into
                    # phase A's queue (head-of-line / clock entanglement)
                    hh = slabp.tile([P, MB, H0P], FP8, tag="slab",
                                    name=f"hh_{r}_{cb}")
                    nc.gpsimd.dma_start(
                        out=hh, in_=h_ar[r][cb * P:(cb + 1) * P, :]
                    )
                    aa = bsp.tile([P, 2, 2, NL], FP8, tag="aa")
                    base = (r * NCORES + cb) * 2 * 2 * NL
                    nc.sync.dma_start(
                        out=aa, in_=a8_d[:, base:base + 2 * 2 * NL]
                    )
                    for mk in range(2):
                        for m in range(MB):
                            nc.tensor.matmul(
                                ps_r[m],
                                aa[:, mk, :, m * P:(m + 1) * P],
                                hh[:, 2 * mk:2 * mk + 2, :],
                                start=(cb == 0 and mk == 0),
                                stop=(cb == NCORES - 1 and mk == 1),
                                perf_mode=DR,
                            )
                for m in range(MB):
                    col = cinv[:, r * MB + m:r * MB + m + 1]
                    if r == 0:
                        nc.vector.tensor_scalar(
                            out=acc[m], in0=ps_r[m],
                            scalar1=col, scalar2=None,
                            op0=mybir.AluOpType.mult,
                        )
                    else:
                        tmp = bnp.tile([P, H0P], F32, tag="btmp")
                        nc.vector.tensor_scalar(
                            out=tmp, in0=ps_r[m],
                            scalar1=col, scalar2=None,
                            op0=mybir.AluOpType.mult,
                        )
                        nc.vector.tensor_add(out=acc[m], in0=acc[m], in1=tmp)

            # ---------------- Phase C: feats -> fc -> BN -> ReLU ----------
            fcw_sb = big.tile([QS, QB, H1], F32, tag="fcw")
            nc.scalar.dma_start(
                out=fcw_sb,
                in_=fcwt_d[:, :].rearrange("(q p) j -> p q j", p=QS),
            )
            ident = big.tile([P, P], F32, tag="ident")
            make_identity(nc, ident)
            biasb = big.tile([P, H0], F32, tag="bias")
            nc.scalar.dma_start(out=biasb, in_=biasb_d[:, :])
            gam = big.tile([P, MB], F32, tag="gam")
            nc.scalar.dma_start(out=gam, in_=gamma_d[:, :])
            bet = big.tile([P, MB], F32, tag="bet")
            nc.scalar.dma_start(out=bet, in_=beta_d[:, :])
            eps_t = big.tile([P, 1], F32, tag="eps")
            nc.vector.memset(eps_t, EPS)

            feats = []
            for m in range(MB):
                f = pp.tile([P, H0], F32, tag="feats", name=f"feats_{m}")
                nc.vector.tensor_add(out=f, in0=acc[m][:, :H0],
                                     in1=rootf[m][:, :H0])
                nc.vector.tensor_add(out=f, in0=f, in1=biasb)
                feats.append(f)

            fT = [pp.tile([P, NL], F32, tag="fT", name=f"fT_{q}")
                  for q in range(QB)]
            for m in range(MB):
                for q in range(QB):
                    pt = psp.tile([P, P], F32, tag="ps", name=f"pt_{m}_{q}")
                    nc.tensor.transpose(
                        pt[:QS, :], feats[m][:, q * QS:(q + 1) * QS], ident
                    )
                    nc.vector.tensor_copy(
                        out=fT[q][:QS, m * P:(m + 1) * P], in_=pt[:QS, :]
                    )

            for m in range(MB):
                pz = psp.tile([P, H1], F32, tag="ps", name=f"pz_{m}")
                for q in range(QB):
                    nc.tensor.matmul(
                        pz,
                        fT[q][:QS, m * P:(m + 1) * P],
                        fcw_sb[:, q, :],
                        start=(q == 0),
                        stop=(q == QB - 1),
                    )
                stats = bnp.tile([P, 6], F32, tag="stats")
                nc.vector.bn_stats(out=stats, in_=pz)
                mv = bnp.tile([P, 2], F32, tag="mv")
                nc.vector.bn_aggr(out=mv, in_=stats)
                rstd = bnp.tile([P, 1], F32, tag="rstd")
                nc.scalar.activation(
                    out=rstd, in_=mv[:, 1:2],
                    func=mybir.ActivationFunctionType.Sqrt,
                    bias=eps_t, scale=1.0,
                )
                nc.vector.reciprocal(out=rstd, in_=rstd)
                g2 = bnp.tile([P, 1], F32, tag="g2")
                nc.vector.tensor_mul(out=g2, in0=rstd, in1=gam[:, m:m + 1])
                zt = bnp.tile([P, H1], F32, tag="zt")
                nc.vector.tensor_scalar(
                    out=zt, in0=pz,
                    scalar1=mv[:, 0:1], scalar2=g2,
                    op0=mybir.AluOpType.subtract, op1=mybir.AluOpType.mult,
                )
                nc.scalar.activation(
                    out=zt, in_=zt,
                    func=mybir.ActivationFunctionType.Relu,
                    bias=bet[:, m:m + 1], scale=1.0,
                )
                nc.scalar.dma_start(out=out_d[m * P:(m + 1) * P, :], in_=zt)

    nc.finalize()
    return nc


def _get_nc(ds_const: float):
    global _NC_CACHE
    if _NC_CACHE is None:
        _NC_CACHE = _build(ds_const)
    return _NC_CACHE


def kernel(**inputs) -> np.ndarray:
    global LAST_RESULTS
    x = np.asarray(inputs["x"], dtype=np.float32)
    basis = np.asarray(inputs["basis"], dtype=np.float32)
    comp = np.asarray(inputs["comp"], dtype=np.float32)
    root = np.asarray(inputs["root"], dtype=np.float32)
    bias_rgcn = np.asarray(inputs["bias_rgcn"], dtype=np.float32)
    fc_w = np.asarray(inputs["fc_w"], dtype=np.float32)
    bn_gamma_u = np.asarray(inputs["bn_gamma_u"], dtype=np.float32)
    bn_beta_u = np.asarray(inputs["bn_beta_u"], dtype=np.float32)
    bn_gamma_i = np.asarray(inputs["bn_gamma_i"], dtype=np.float32)
    bn_beta_i = np.asarray(inputs["bn_beta_i"], dtype=np.float32)
    edge_index = np.asarray(inputs["edge_index"]).astype(np.int64)
    edge_type = np.asarray(inputs["edge_type"]).astype(np.int64)

    src, dst = edge_index[0], edge_index[1]
    et = edge_type

    # W[r] = sum_b comp[r,b] basis[b]
    W = np.tensordot(comp, basis, axes=([1], [0]))          # [R, N, H0]

    # fp8 power-of-2 scales
    SX = _pow2_scale(float(np.abs(x).max()), 224.0)
    SW = _pow2_scale(float(np.abs(W).max()), 224.0)
    # exact h absmax (host matmul) with 2x headroom for fp8-vs-fp32 drift
    Wflat = W.transpose(1, 0, 2).reshape(N, R * H0)
    h_abs = float(np.abs(x @ Wflat).max())
    SH = _pow2_scale(h_abs, 100.0)
    DS = SH / (SX * SW)

    # x8[p, kb2, i, s] = q8(x[s, kb2*256+i*128+p] * SX)   (s per-core below)
    x8_full = (_q8(x.T, SX)                     # [k, s]
               .reshape(KB2, 2, P, N)           # [kb2, i, p, s]
               .transpose(2, 0, 1, 3))          # [p, kb2, i, s]

    # w8[p, n, kb2, i, j(512 padded)]
    w8 = np.zeros((P, R, KB2, 2, H0P), dtype=NP8)
    w8[:, :, :, :, :H0] = (
        _q8(W.transpose(1, 0, 2), SW)           # [k, n, j]
        .reshape(KB2, 2, P, R, H0)              # [kb2, i, p, n, j]
        .transpose(2, 3, 0, 1, 4))              # [p, n, kb2, i, j]
    w8 = w8.reshape(P, R * KB2 * 2 * H0P)

    # bf16 x / root for the root block
    xb_full = (x.T.astype(ml_dtypes.bfloat16)   # [k, s]
               .reshape(KBR, P, N)              # [kb, p, s]
               .transpose(1, 0, 2))             # [p, kb, s]
    rw = np.zeros((P, KBR, H0P), dtype=ml_dtypes.bfloat16)
    rw[:, :, :H0] = (root.astype(ml_dtypes.bfloat16)
                     .reshape(KBR, P, H0)
                     .transpose(1, 0, 2))
    rw = rw.reshape(P, KBR * H0P)

    # edge-multiplicity adjacency (exact small ints in fp8) + mean counts
    lin = (et * N + src) * np.int64(N) + dst
    cnts = np.bincount(lin, minlength=R * N * N)
    a8_full = (cnts.astype(np.float32).astype(NP8)
               .reshape(R, NCORES, 2, 2, P, N)  # [r, cb, mk, i, p, dst]
               .transpose(4, 0, 1, 2, 3, 5))    # [p, r, cb, mk, i, dst]

    cnt = np.bincount(dst * R + et, minlength=N * R).astype(np.float64)
    cinv_full = (1.0 / (np.maximum(cnt, 1.0) * SH)).astype(np.float32)
    cinv_full = cinv_full.reshape(N, R)         # [dst, r]

    fcwt = np.ascontiguousarray(fc_w.T)
    biasb = np.ascontiguousarray(
        np.broadcast_to(bias_rgcn, (P, H0)), dtype=np.float32)
    gamma_all = np.concatenate([bn_gamma_u, bn_gamma_i])
    beta_all = np.concatenate([bn_beta_u, bn_beta_i])

    in_maps = []
    for c in range(NCORES):
        sl = slice(c * NL, (c + 1) * NL)
        cinv_c = cinv_full[sl].reshape(MB, P, R).transpose(1, 2, 0)
        in_maps.append({
            "x8": np.ascontiguousarray(
                x8_full[:, :, :, sl]).reshape(P, KB2 * 2 * NL),
            "w8": w8,
            "xb": np.ascontiguousarray(
                xb_full[:, :, sl]).reshape(P, KBR * NL),
            "rw": rw,
            "a8": np.ascontiguousarray(
                a8_full[:, :, :, :, :, sl]).reshape(P, R * NCORES * 4 * NL),
            "cinv": np.ascontiguousarray(cinv_c).reshape(P, R * MB),
            "fcwt": fcwt,
            "biasb": biasb,
            "gamma": np.ascontiguousarray(gamma_all[sl].reshape(MB, P).T),
            "beta": np.ascontiguousarray(beta_all[sl].reshape(MB, P).T),
        })

    nc = _get_nc(DS)
    res = run_bass_kernel_spmd(
        nc, in_maps, core_ids=list(range(NCORES)), trace=TRACE,
    )
    LAST_RESULTS = res

    z = np.concatenate([res.results[c]["out"] for c in range(NCORES)], axis=0)
    return np.stack([z[:U], z[U:]], axis=0)


# revision 8
# speedup vs baseline: 1.5206x; 1.1176x over previous
"""GCEncoder (RGCN basis-decomposition conv + mean aggregation + Dense/BN/ReLU)
as a Bass/Tile kernel on 8 Trainium2 NeuronCores.

Math (reference):
  W[r]  = sum_b comp[r,b] * basis[b]                    [R, N, H0]
  h[r]  = x @ W[r]                                      [R, N, H0]
  agg[d] = sum_r (1/cnt[d,r]) * sum_{e: dst=d, type=r} h[r, src_e]
  feats = agg + x @ root + bias
  z     = feats @ fc_w.T ; per-row batchnorm over H1 + gamma/beta + relu
  out   = (z[:U], z[U:]) stacked -> [2, U, H1]

Device strategy (per core c of 8, 512 node-rows each), fp8-DoubleRow:
  Phase A: h_c = x[rows] @ W[r] for r=0..4 as fp8e4 DoubleRow matmuls
           (256-deep contraction per instruction, 2 fp8 MACs/cell/cycle);
           per-relation h blocks are scaled to fp8, drained on the scalar
           DMA queue (kept free of bulk traffic so the AllGather chain
           starts as early as possible) and AllGathered immediately.
           The root block x @ root stays bf16 for accuracy.
  Phase B: agg via dense 0/1-multiplicity adjacency matmul in fp8
           DoubleRow (adjacency counts are exact in fp8).  Per-relation
           PSUM accumulation; the 1/cnt mean normalization is applied in
           exact fp32 on the vector engine per relation.
  Phase C: feats = agg + root_part + bias; PE-transpose; z = feats @
           fc_w.T; per-row BN (bn_stats/bn_aggr) + gamma/beta + ReLU.

All fp8 scales are powers of two so scaling/descale is exact.  H0=500
is padded to 512 (zero padding) so DoubleRow pair-steps are 16B-aligned
and PSUM tiles are exactly one bank.
"""
import numpy as np
import ml_dtypes

import concourse.bacc as bacc
import concourse.mybir as mybir
import concourse.tile as tile
from concourse.bass_utils import run_bass_kernel_spmd
from concourse.masks import make_identity

P = 128
NCORES = 8
N = 4096          # nodes
U = 2048          # users
R = 5             # relations
H0 = 500
H0P = 512         # padded H0
H1 = 75
EPS = 1e-5

NL = N // NCORES              # 512 node rows per core
KB2 = N // 256                # 16 DoubleRow contraction blocks (phase A)
KBR = N // P                  # 32 bf16 contraction blocks (root)
MB = NL // P                  # 4 M-tiles per core
QB = 4                        # H0 chunks for transpose/fc
QS = H0 // QB                 # 125

F32 = mybir.dt.float32
BF16 = mybir.dt.bfloat16
FP8 = mybir.dt.float8e4
DR = mybir.MatmulPerfMode.DoubleRow
NP8 = ml_dtypes.float8_e4m3   # TRN fp8_e4m3 (bias 7, max 240)

# test hooks
TRACE = False
LAST_RESULTS = None
_NC_CACHE = None


def _pow2_scale(absmax: float, cap: float) -> float:
    return float(2.0 ** np.floor(np.log2(cap / max(absmax, 1e-30))))


def _q8(a: np.ndarray, scale: float) -> np.ndarray:
    return np.clip(a * scale, -240.0, 240.0).astype(NP8)


def _build(ds_const: float):
    nc = bacc.Bacc("TRN2", target_bir_lowering=False, debug=False,
                   num_devices=NCORES)

    # host-swizzled inputs; layouts noted as [partition, free...]
    # x8[p, kb2*1024 + i*512 + s] = q8(x[rows s][k=kb2*256+i*128+p] * SX)
    x8_d = nc.dram_tensor("x8", [P, KB2 * 2 * NL], FP8, kind="ExternalInput")
    # w8[p, ((n*16+kb2)*2+i)*512 + j] = q8(W[n][kb2*256+i*128+p, j] * SW)
    w8_d = nc.dram_tensor("w8", [P, R * KB2 * 2 * H0P], FP8,
                          kind="ExternalInput")
    # xb[p, kb*512 + s] = bf16(x[rows s][kb*128+p])
    xb_d = nc.dram_tensor("xb", [P, KBR * NL], BF16, kind="ExternalInput")
    # rw[p, kb*512 + j] = bf16(root[kb*128+p, j])  (j >= 500 zero)
    rw_d = nc.dram_tensor("rw", [P, KBR * H0P], BF16, kind="ExternalInput")
    # a8[p, (((r*8+cb)*2+mk)*2+i)*512 + d] =
    #   edgecount[r, src=cb*512+(2mk+i)*128+p, dst=core*512+d]
    a8_d = nc.dram_tensor("a8", [P, R * NCORES * 2 * 2 * NL], FP8,
                          kind="ExternalInput")
    # cinv[p, r*4+m] = 1/(max(cnt[core*512+m*128+p, r],1) * SH)
    cinv_d = nc.dram_tensor("cinv", [P, R * MB], F32, kind="ExternalInput")
    fcwt_d = nc.dram_tensor("fcwt", [H0, H1], F32, kind="ExternalInput")
    biasb_d = nc.dram_tensor("biasb", [P, H0], F32, kind="ExternalInput")
    gamma_d = nc.dram_tensor("gamma", [P, MB], F32, kind="ExternalInput")
    beta_d = nc.dram_tensor("beta", [P, MB], F32, kind="ExternalInput")
    out_d = nc.dram_tensor("out", [NL, H1], F32, kind="ExternalOutput")

    with tile.TileContext(nc) as tc:
        with (
            tc.tile_pool(name="big", bufs=1) as big,
            tc.tile_pool(name="slab", bufs=3) as slabp,
            tc.tile_pool(name="hh", bufs=4) as hhp,
            tc.tile_pool(name="io", bufs=2) as iop,
            tc.tile_pool(name="bstream", bufs=4) as bsp,
            tc.tile_pool(name="persist", bufs=4) as pp,
            tc.tile_pool(name="bn", bufs=4) as bnp,
            tc.tile_pool(name="ps", bufs=8, space="PSUM") as psp,
            tc.tile_pool(name="dram", bufs=1, space="DRAM") as dramp,
        ):
            # ---------------- Phase A: h[r] = x_rows @ W[r] (fp8 DR) ------
            pre_slab = slabp.tile([P, KB2, 2, H0P], FP8, tag="slab",
                                  name="slab00")
            nc.scalar.dma_start(out=pre_slab, in_=w8_d[:, :KB2 * 2 * H0P])
            x8_sb = big.tile([P, KB2, 2, NL], FP8, tag="x8")
            for ch in range(4):
                eng = nc.sync if ch < 2 else nc.scalar
                eng.dma_start(
                    out=x8_sb[:, ch * 4:(ch + 1) * 4, :, :],
                    in_=x8_d[:, ch * 4 * 2 * NL:(ch + 1) * 4 * 2 * NL],
                )
            cinv = big.tile([P, R * MB], F32, tag="cinv")
            nc.scalar.dma_start(out=cinv, in_=cinv_d[:, :])

            h_cr = [dramp.tile([P, MB * H0P], FP8, tag="h_c", name=f"h_c{r}")
                    for r in range(R)]
            h_ar = [dramp.tile([NCORES * P, MB * H0P], FP8, tag="h_a",
                               addr_space="Shared", name=f"h_a{r}")
                    for r in range(R)]

            for n in range(R):
                if n == 0:
                    slab = pre_slab
                else:
                    slab = slabp.tile([P, KB2, 2, H0P], FP8, tag="slab")
                    base = n * KB2 * 2 * H0P
                    nc.sync.dma_start(
                        out=slab, in_=w8_d[:, base:base + KB2 * 2 * H0P]
                    )
                ps_n = [psp.tile([P, H0P], F32, tag="ps",
                                 name=f"psA_{n}_{m}") for m in range(MB)]
                for kb2 in range(KB2):
                    for m in range(MB):
                        nc.tensor.matmul(
                            ps_n[m],
                            x8_sb[:, kb2, :, m * P:(m + 1) * P],
                            slab[:, kb2, :, :],
                            start=(kb2 == 0),
                            stop=(kb2 == KB2 - 1),
                            perf_mode=DR,
                        )
                h8blk = iop.tile([P, MB, H0P], FP8, tag="hout")
                for m in range(MB):
                    # h8 = psum * SH/(SX*SW), cast to fp8
                    nc.vector.tensor_scalar(
                        out=h8blk[:, m, :], in0=ps_n[m],
                        scalar1=ds_const, scalar2=None,
                        op0=mybir.AluOpType.mult,
                    )
                nc.scalar.dma_start(out=h_cr[n][:, :], in_=h8blk)
                nc.gpsimd.collective_compute(
                    "AllGather",
                    mybir.AluOpType.bypass,
                    replica_groups=[list(range(NCORES))],
                    ins=[h_cr[n][:, :]],
                    outs=[h_ar[n][:, :]],
                )

            # xb for the root block (issued after all h drains so it
            # cannot delay them in the scalar DMA queue)
            xb_sb = big.tile([P, KBR, NL], BF16, tag="xb")
            for ch in range(4):
                nc.scalar.dma_start(
                    out=xb_sb[:, ch * 8:(ch + 1) * 8, :],
                    in_=xb_d[:, ch * 8 * NL:(ch + 1) * 8 * NL],
                )

            # ---------------- Phase A2: root block (bf16) -----------------
            rootf = []
            ps_rt = [psp.tile([P, H0P], F32, tag="ps", name=f"psR_{m}")
                     for m in range(MB)]
            for kh in range(2):
                rslab = slabp.tile([P, KBR // 2, H0P], BF16, tag="slab",
                                   name=f"rslab{kh}")
                base = kh * (KBR // 2) * H0P
                nc.sync.dma_start(
                    out=rslab, in_=rw_d[:, base:base + (KBR // 2) * H0P]
                )
                for k in range(KBR // 2):
                    kb = kh * (KBR // 2) + k
                    for m in range(MB):
                        nc.tensor.matmul(
                            ps_rt[m],
                            xb_sb[:, kb, m * P:(m + 1) * P],
                            rslab[:, k, :],
                            start=(kb == 0),
                            stop=(kb == KBR - 1),
                        )
            for m in range(MB):
                rf = pp.tile([P, H0P], F32, tag="rootf", name=f"rootf_{m}")
                nc.vector.tensor_copy(out=rf, in_=ps_rt[m])
                rootf.append(rf)

            # ---------------- Phase B: agg (fp8 DR, per-relation) ---------
            acc = [pp.tile([P, H0P], F32, tag="acc", name=f"acc_{m}")
                   for m in range(MB)]
            for r in range(R):
                ps_r = [psp.tile([P, H0P], F32, tag="ps",
                                 name=f"psB_{r}_{m}") for m in range(MB)]
                for cb in range(NCORES):
                    hh = hhp.tile([P, MB, H0P], FP8, tag="hh",
                                  name=f"hh_{r}_{cb}")
                    nc.gpsimd.dma_start(
                        out=hh, in_=h_ar[r][cb * P:(cb + 1) * P, :]
                    )
                    aa = bsp.tile([P, 2, 2, NL], FP8, tag="aa")
                    base = (r * NCORES + cb) * 2 * 2 * NL
                    nc.sync.dma_start(
                        out=aa, in_=a8_d[:, base:base + 2 * 2 * NL]
                    )
                    for mk in range(2):
                        for m in range(MB):
                            nc.tensor.matmul(
                                ps_r[m],
                                aa[:, mk, :, m * P:(m + 1) * P],
                                hh[:, 2 * mk:2 * mk + 2, :],
                                start=(cb == 0 and mk == 0),
                                stop=(cb == NCORES - 1 and mk == 1),
                                perf_mode=DR,
                            )
                for m in range(MB):
                    col = cinv[:, r * MB + m:r * MB + m + 1]
                    if r == 0:
                        nc.vector.tensor_scalar(
                            out=acc[m], in0=ps_r[m],
                            scalar1=col, scalar2=None,
                            op0=mybir.AluOpType.mult,
                        )
                    else:
                        tmp = bnp.tile([P, H0P], F32, tag="btmp")
                        nc.vector.tensor_scalar(
                            out=tmp, in0=ps_r[m],
                            scalar1=col, scalar2=None,
                            op0=mybir.AluOpType.mult,
                        )
                        nc.vector.tensor_add(out=acc[m], in0=acc[m], in1=tmp)

            # ---------------- Phase C: feats -> fc -> BN -> ReLU ----------
            fcw_sb = big.tile([QS, QB, H1], F32, tag="fcw")
            nc.scalar.dma_start(
                out=fcw_sb,
                in_=fcwt_d[:, :].rearrange("(q p) j -> p q j", p=QS),
            )
            ident = big.tile([P, P], F32, tag="ident")
            make_identity(nc, ident)
            biasb = big.tile([P, H0], F32, tag="bias")
            nc.scalar.dma_start(out=biasb, in_=biasb_d[:, :])
            gam = big.tile([P, MB], F32, tag="gam")
            nc.scalar.dma_start(out=gam, in_=gamma_d[:, :])
            bet = big.tile([P, MB], F32, tag="bet")
            nc.scalar.dma_start(out=bet, in_=beta_d[:, :])
            eps_t = big.tile([P, 1], F32, tag="eps")
            nc.vector.memset(eps_t, EPS)

            feats = []
            for m in range(MB):
                f = pp.tile([P, H0], F32, tag="feats", name=f"feats_{m}")
                nc.vector.tensor_add(out=f, in0=acc[m][:, :H0],
                                     in1=rootf[m][:, :H0])
                nc.vector.tensor_add(out=f, in0=f, in1=biasb)
                feats.append(f)

            fT = [pp.tile([P, NL], F32, tag="fT", name=f"fT_{q}")
                  for q in range(QB)]
            for m in range(MB):
                for q in range(QB):
                    pt = psp.tile([P, P], F32, tag="ps", name=f"pt_{m}_{q}")
                    nc.tensor.transpose(
                        pt[:QS, :], feats[m][:, q * QS:(q + 1) * QS], ident
                    )
                    nc.vector.tensor_copy(
                        out=fT[q][:QS, m * P:(m + 1) * P], in_=pt[:QS, :]
                    )

            for m in range(MB):
                pz = psp.tile([P, H1], F32, tag="ps", name=f"pz_{m}")
                for q in range(QB):
                    nc.tensor.matmul(
                        pz,
                        fT[q][:QS, m * P:(m + 1) * P],
                        fcw_sb[:, q, :],
                        start=(q == 0),
                        stop=(q == QB - 1),
                    )
                stats = bnp.tile([P, 6], F32, tag="stats")
                nc.vector.bn_stats(out=stats, in_=pz)
                mv = bnp.tile([P, 2], F32, tag="mv")
                nc.vector.bn_aggr(out=mv, in_=stats)
                rstd = bnp.tile([P, 1], F32, tag="rstd")
                nc.scalar.activation(
                    out=rstd, in_=mv[:, 1:2],
                    func=mybir.ActivationFunctionType.Sqrt,
                    bias=eps_t, scale=1.0,
                )
                nc.vector.reciprocal(out=rstd, in_=rstd)
                g2 = bnp.tile([P, 1], F32, tag="g2")
                nc.vector.tensor_mul(out=g2, in0=rstd, in1=gam[:, m:m + 1])
                zt = bnp.tile([P, H1], F32, tag="zt")
                nc.vector.tensor_scalar(
                    out=zt, in0=pz,
                    scalar1=mv[:, 0:1], scalar2=g2,
                    op0=mybir.AluOpType.subtract, op1=mybir.AluOpType.mult,
                )
                nc.scalar.activation(
                    out=zt, in_=zt,
                    func=mybir.ActivationFunctionType.Relu,
                    bias=bet[:, m:m + 1], scale=1.0,
                )
                nc.scalar.dma_start(out=out_d[m * P:(m + 1) * P, :], in_=zt)

    nc.finalize()
    return nc


def _get_nc(ds_const: float):
    global _NC_CACHE
    if _NC_CACHE is None:
        _NC_CACHE = _build(ds_const)
    return _NC_CACHE


def kernel(**inputs) -> np.ndarray:
    global LAST_RESULTS
    x = np.asarray(inputs["x"], dtype=np.float32)
    basis = np.asarray(inputs["basis"], dtype=np.float32)
    comp = np.asarray(inputs["comp"], dtype=np.float32)
    root = np.asarray(inputs["root"], dtype=np.float32)
    bias_rgcn = np.asarray(inputs["bias_rgcn"], dtype=np.float32)
    fc_w = np.asarray(inputs["fc_w"], dtype=np.float32)
    bn_gamma_u = np.asarray(inputs["bn_gamma_u"], dtype=np.float32)
    bn_beta_u = np.asarray(inputs["bn_beta_u"], dtype=np.float32)
    bn_gamma_i = np.asarray(inputs["bn_gamma_i"], dtype=np.float32)
    bn_beta_i = np.asarray(inputs["bn_beta_i"], dtype=np.float32)
    edge_index = np.asarray(inputs["edge_index"]).astype(np.int64)
    edge_type = np.asarray(inputs["edge_type"]).astype(np.int64)

    src, dst = edge_index[0], edge_index[1]
    et = edge_type

    # W[r] = sum_b comp[r,b] basis[b]
    W = np.tensordot(comp, basis, axes=([1], [0]))          # [R, N, H0]

    # fp8 power-of-2 scales
    SX = _pow2_scale(float(np.abs(x).max()), 224.0)
    SW = _pow2_scale(float(np.abs(W).max()), 224.0)
    # exact h absmax (host matmul) with 2x headroom for fp8-vs-fp32 drift
    Wflat = W.transpose(1, 0, 2).reshape(N, R * H0)
    h_abs = float(np.abs(x @ Wflat).max())
    SH = _pow2_scale(h_abs, 100.0)
    DS = SH / (SX * SW)

    # x8[p, kb2, i, s] = q8(x[s, kb2*256+i*128+p] * SX)   (s per-core below)
    x8_full = (_q8(x.T, SX)                     # [k, s]
               .reshape(KB2, 2, P, N)           # [kb2, i, p, s]
               .transpose(2, 0, 1, 3))          # [p, kb2, i, s]

    # w8[p, n, kb2, i, j(512 padded)]
    w8 = np.zeros((P, R, KB2, 2, H0P), dtype=NP8)
    w8[:, :, :, :, :H0] = (
        _q8(W.transpose(1, 0, 2), SW)           # [k, n, j]
        .reshape(KB2, 2, P, R, H0)              # [kb2, i, p, n, j]
        .transpose(2, 3, 0, 1, 4))              # [p, n, kb2, i, j]
    w8 = w8.reshape(P, R * KB2 * 2 * H0P)

    # bf16 x / root for the root block
    xb_full = (x.T.astype(ml_dtypes.bfloat16)   # [k, s]
               .reshape(KBR, P, N)              # [kb, p, s]
               .transpose(1, 0, 2))             # [p, kb, s]
    rw = np.zeros((P, KBR, H0P), dtype=ml_dtypes.bfloat16)
    rw[:, :, :H0] = (root.astype(ml_dtypes.bfloat16)
                     .reshape(KBR, P, H0)
                     .transpose(1, 0, 2))
    rw = rw.reshape(P, KBR * H0P)

    # edge-multiplicity adjacency (exact small ints in fp8) + mean counts
    lin = (et * N + src) * np.int64(N) + dst
    cnts = np.bincount(lin, minlength=R * N * N)
    a8_full = (cnts.astype(np.float32).astype(NP8)
               .reshape(R, NCORES, 2, 2, P, N)  # [r, cb, mk, i, p, dst]
               .transpose(4, 0, 1, 2, 3, 5))    # [p, r, cb, mk, i, dst]

    cnt = np.bincount(dst * R + et, minlength=N * R).astype(np.float64)
    cinv_full = (1.0 / (np.maximum(cnt, 1.0) * SH)).astype(np.float32)
    cinv_full = cinv_full.reshape(N, R)         # [dst, r]

    fcwt = np.ascontiguousarray(fc_w.T)
    biasb = np.ascontiguousarray(
        np.broadcast_to(bias_rgcn, (P, H0)), dtype=np.float32)
    gamma_all = np.concatenate([bn_gamma_u, bn_gamma_i])
    beta_all = np.concatenate([bn_beta_u, bn_beta_i])

    in_maps = []
    for c in range(NCORES):
        sl = slice(c * NL, (c + 1) * NL)
        cinv_c = cinv_full[sl].reshape(MB, P, R).transpose(1, 2, 0)
        in_maps.append({
            "x8": np.ascontiguousarray(
                x8_full[:, :, :, sl]).reshape(P, KB2 * 2 * NL),
            "w8": w8,
            "xb": np.ascontiguousarray(
                xb_full[:, :, sl]).reshape(P, KBR * NL),
            "rw": rw,
            "a8": np.ascontiguousarray(
                a8_full[:, :, :, :, :, sl]).reshape(P, R * NCORES * 4 * NL),
            "cinv": np.ascontiguousarray(cinv_c).reshape(P, R * MB),
            "fcwt": fcwt,
            "biasb": biasb,
            "gamma": np.ascontiguousarray(gamma_all[sl].reshape(MB, P).T),
            "beta": np.ascontiguousarray(beta_all[sl].reshape(MB, P).T),
        })

    nc = _get_nc(DS)
    res = run_bass_kernel_spmd(
        nc, in_maps, core_ids=list(range(NCORES)), trace=TRACE,
    )
    LAST_RESULTS = res

    z = np.concatenate([res.results[c]["out"] for c in range(NCORES)], axis=0)
    return np.stack([z[:U], z[U:]], axis=0)
